# revision 6
# baseline (speedup 1.0000x reference)
"""CfC head (3 stacked CfC cells, seq_len=1, h0=0) on 8 TRN2 NeuronCores.

Math (per cell, zero initial hidden state, ts=1):
    ff1 = tanh(x @ (Wf1*mask)[:in] + bf1)
    ff2 = tanh(x @ (Wf2*mask)[:in] + bf2)
    s   = sigmoid(x @ (Wtb - Wta)[:in] + (btb - bta))
    out = ff1 + s * (ff2 - ff1)

Because h0 == 0 only the first in_dim rows of each weight matter, the
sparsity mask folds into the weights, and t_a/t_b fold into a single
matmul.  All of that is O(params) host-side prep; the O(B) work runs on
the NeuronCores, data-parallel over the batch.

Device layout is feature-major ([feat, batch]); x is transposed on the
host so every DMA is contiguous per partition.
"""

import numpy as np

import concourse.bass as bass
import concourse.tile as tile
from concourse import mybir
from concourse.bass_utils import run_bass_kernel_spmd

# ---------------------------------------------------------------- dims
INPUT_DIM, INTER, COMMAND, MOTOR = 74, 269, 179, 64
BATCH = 65536
N_CORES = 8
B_CORE = BATCH // N_CORES          # 8192 rows per core
G = 2048                           # batch columns per pipeline slice
N_MM = 512                         # moving free dim per matmul (fp32 max)
SLICES = B_CORE // G

LAYER_DIMS = [(INPUT_DIM, INTER), (INTER, COMMAND), (COMMAND, MOTOR)]
MATS = ("f1", "f2", "t")
F32 = mybir.dt.float32
F32R = mybir.dt.float32r


def _splits(n, step=128):
    """[(start, size), ...] covering n in chunks of <=step."""
    return [(i, min(step, n - i)) for i in range(0, n, step)]


# Elementwise-instance schedule: one entry per (layer, mtile); the bias
# pack column index for (layer, mat, mtile) is shared between host pack
# and device program via this table.
BIAS_COLS = {}
_c = 0
for _l, (_in, _hid) in enumerate(LAYER_DIMS):
    for _m, _ in enumerate(_splits(_hid)):
        for _mat in MATS:
            BIAS_COLS[(_l, _mat, _m)] = _c
            _c += 1
N_BIAS_COLS = _c


# ---------------------------------------------- walrus sync-wait workaround
def _split_multi_waits(nc):
    """This walrus build accepts only ONE sync-wait command per
    instruction.  Tile attaches one wait per outstanding proc, so after
    scheduling, hoist every excess wait onto a single-wait NOP emitted
    just before the instruction on the same engine (engine queues are
    in-order, so the waits still all complete before it executes)."""
    import bass_rust as _br

    for fn in nc.m.functions:
        for blk in fn.blocks:
            out = []
            changed = False
            for inst in blk.instructions:
                si = inst.sync_info
                if si is not None and len(si.on_wait) > 1:
                    waits = list(si.on_wait)
                    for j, w in enumerate(waits[:-1]):
                        carrier = mybir.InstNoOp(
                            name=f"{inst.name}-sw{j}", engine=inst.engine
                        )
                        carrier.sync_info = _br.SyncInfo(on_wait=[w], on_update=[])
                        out.append(carrier)
                    inst.sync_info = _br.SyncInfo(
                        on_wait=[waits[-1]], on_update=list(si.on_update)
                    )
                    changed = True
                out.append(inst)
            if changed:
                blk.instructions = out
    return nc


# ---------------------------------------------------------------- device
def _build_nc():
    nc = bass.Bass(target_bir_lowering=False)

    xT = nc.dram_tensor("xT", [INPUT_DIM, B_CORE], F32R, kind="ExternalInput")
    w_dram = {}
    for l, (ind, hid) in enumerate(LAYER_DIMS):
        for mat in MATS:
            w_dram[(l, mat)] = nc.dram_tensor(
                f"w{l}{mat}", [ind, hid], F32R, kind="ExternalInput"
            )
    bias_dram = nc.dram_tensor("biases", [128, N_BIAS_COLS], F32, kind="ExternalInput")
    outT = nc.dram_tensor("outT", [MOTOR, B_CORE], F32, kind="ExternalOutput")

    TANH = mybir.ActivationFunctionType.Tanh
    SIGM = mybir.ActivationFunctionType.Sigmoid

    with tile.TileContext(nc) as tc:
        with (
            tc.tile_pool(name="consts", bufs=1) as consts,
            tc.tile_pool(name="xt", bufs=2) as xt_pool,
            tc.tile_pool(name="act0", bufs=2) as o0_pool,
            tc.tile_pool(name="act1", bufs=2) as o1_pool,
            tc.tile_pool(name="act2", bufs=2) as o2_pool,
            tc.tile_pool(name="ff", bufs=2) as ff_pool,
            tc.tile_pool(name="ps", bufs=2, space="PSUM") as ps_pool,
        ):
            # ---- constants: weights as lhsT tiles [K, M], packed biases
            bias_sb = consts.tile([128, N_BIAS_COLS], F32, tag="bias")
            nc.sync.dma_start(out=bias_sb[:], in_=bias_dram[:])

            wt = {}  # (l, mat, mtile, ktile) -> sbuf AP [Ksz, Msz]
            for l, (ind, hid) in enumerate(LAYER_DIMS):
                for mat in MATS:
                    for mi, (m0, msz) in enumerate(_splits(hid)):
                        for ki, (k0, ksz) in enumerate(_splits(ind)):
                            w = consts.tile(
                                [ksz, msz], F32R, tag=f"w{l}{mat}m{mi}k{ki}"
                            )
                            nc.sync.dma_start(
                                out=w[:],
                                in_=w_dram[(l, mat)][k0 : k0 + ksz, m0 : m0 + msz],
                            )
                            wt[(l, mat, mi, ki)] = w

            # ---- per-slice pipeline
            for s in range(SLICES):
                c0 = s * G

                xt = xt_pool.tile([INPUT_DIM, G], F32R, tag="xt")
                nc.sync.dma_start(out=xt[:], in_=xT[:, c0 : c0 + G])

                def layer(l, rhs_tiles, out_pool, out_tag, out_dtype=F32R):
                    """rhs_tiles: list of (ktile_ap, ksz) feature-major
                    [ksz, G] inputs.  Returns same for this layer's out."""
                    ind, hid = LAYER_DIMS[l]
                    outs = []
                    for mi, (m0, msz) in enumerate(_splits(hid)):
                        ff = {}
                        for mat in MATS:
                            ps = ps_pool.tile([msz, G], F32, tag="ps")
                            nk = len(rhs_tiles)
                            for n in range(G // N_MM):
                                nsl = slice(n * N_MM, (n + 1) * N_MM)
                                for ki, (rhs, ksz) in enumerate(rhs_tiles):
                                    nc.tensor.matmul(
                                        ps[:, nsl],
                                        wt[(l, mat, mi, ki)][:],
                                        rhs[:, nsl],
                                        start=(ki == 0),
                                        stop=(ki == nk - 1),
                                    )
                            f = ff_pool.tile([msz, G], F32, tag=f"ff_{mat}")
                            bcol = BIAS_COLS[(l, mat, mi)]
                            nc.scalar.activation(
                                out=f[:],
                                in_=ps[:],
                                func=SIGM if mat == "t" else TANH,
                                bias=bias_sb[:msz, bcol : bcol + 1],
                            )
                            ff[mat] = f
                        # out = ff1 + s*(ff2-ff1)
                        d = ff_pool.tile([msz, G], F32, tag="d")
                        nc.vector.tensor_sub(d[:], ff["f2"][:], ff["f1"][:])
                        nc.vector.tensor_mul(ff["f2"][:], ff["t"][:], d[:])
                        o = out_pool.tile([msz, G], out_dtype, tag=f"{out_tag}{mi}")
                        nc.gpsimd.tensor_add(o[:], ff["f1"][:], ff["f2"][:])
                        outs.append((o, msz))
                    return outs

                o0 = layer(0, [(xt, INPUT_DIM)], o0_pool, "o0_")
                o1 = layer(1, o0, o1_pool, "o1_")
                o2 = layer(2, o1, o2_pool, "o2_", out_dtype=F32)

                assert len(o2) == 1
                nc.sync.dma_start(out=outT[:, c0 : c0 + G], in_=o2[0][0][:])

    return _split_multi_waits(nc)


_NC_CACHE = None


def _get_nc():
    global _NC_CACHE
    if _NC_CACHE is None:
        _NC_CACHE = _build_nc()
    return _NC_CACHE


# ------------------------------------------------------------------ host
def _prep_host_inputs(inputs):
    """Fold masks / t-diff / per-core shards.  Returns per-core in_maps."""
    f32 = np.float32
    common = {}
    for l, (ind, hid) in enumerate(LAYER_DIMS):
        m = inputs[f"mask_{l}"][:ind].astype(f32)
        common[f"w{l}f1"] = np.ascontiguousarray(
            inputs[f"Wf1_{l}"][:ind] * m, dtype=f32
        )
        common[f"w{l}f2"] = np.ascontiguousarray(
            inputs[f"Wf2_{l}"][:ind] * m, dtype=f32
        )
        common[f"w{l}t"] = np.ascontiguousarray(
            inputs[f"Wtb_{l}"][:ind] - inputs[f"Wta_{l}"][:ind], dtype=f32
        )
    biases = np.zeros((128, N_BIAS_COLS), dtype=f32)
    for l, (ind, hid) in enumerate(LAYER_DIMS):
        bf1, bf2 = inputs[f"bf1_{l}"], inputs[f"bf2_{l}"]
        bt = inputs[f"btb_{l}"] - inputs[f"bta_{l}"]
        for mi, (m0, msz) in enumerate(_splits(hid)):
            for mat, b in (("f1", bf1), ("f2", bf2), ("t", bt)):
                biases[:msz, BIAS_COLS[(l, mat, mi)]] = b[m0 : m0 + msz]
    common["biases"] = biases

    xT = np.ascontiguousarray(np.asarray(inputs["x"], dtype=f32).T)  # [74, B]
    in_maps = []
    for c in range(N_CORES):
        m = dict(common)
        m["xT"] = np.ascontiguousarray(xT[:, c * B_CORE : (c + 1) * B_CORE])
        in_maps.append(m)
    return in_maps


def run(inputs, trace=False, **kw):
    """Run on hardware; returns (out [BATCH, MOTOR] fp32, BassKernelResults)."""
    nc = _get_nc()
    in_maps = _prep_host_inputs(inputs)
    res = run_bass_kernel_spmd(
        nc, in_maps, core_ids=list(range(N_CORES)), trace=trace, **kw
    )
    out = np.empty((BATCH, MOTOR), dtype=np.float32)
    for c in range(N_CORES):
        out[c * B_CORE : (c + 1) * B_CORE, :] = res.results[c]["outT"].T
    return out, res


def kernel(**inputs) -> np.ndarray:
    out, _ = run(inputs, trace=False)
    return out
